# revision 3
# baseline (speedup 1.0000x reference)
"""Trainium2 Bass kernel for the patch-pooling + FF model.

Reference math (per (b, d) slice, x_s = x[b, d] of shape (N=254, L=16)):
    c      = x_s @ W_enc + b_enc + pos_embed          # (N, H)
    csum   = sum_n c                                  # (H,)
    score  = c @ csum (/ sqrt(H), cancels)            # (N,)
    weight = score / sum_n |score|                    # (N,)
    pooled = x_s.T @ weight                           # (L,)
    h      = leaky_relu(pooled @ W1 + b1, 0.2)        # (H,)
    out    = h @ W2 + b2                              # (P,)
returns (out.transpose(0, 2, 1), x)

Key algebra: score_n = c_n . csum = x_n . (W_enc @ csum)
                     + (pos_embed_n + b_enc) . csum
so c (B,D,N,H) is never materialized. b_enc rides inside pos_embedT.

Sharding: data-parallel over batch, 4 batches (256 (b,d) slices) per core.
Layout: 128 slices on partitions, (n, l) = 4064 f32 on the free dim.
GPSIMD is avoided entirely (it shares SBUF ports with DVE - exclusive
lock, so "offloading" there just stalls DVE). Work splits DVE/ScalarE:
ScalarE casts x to bf16 and sums most xs l-planes via activation+accum;
DVE runs the bf16 products (2x mode) and the 1x reduces; PE does the
small matmuls.
"""

import numpy as np

import concourse.bacc as bacc
import concourse.bass as bass
import concourse.tile as tile
from concourse import mybir
from concourse.bass_utils import run_bass_kernel_spmd

F32 = mybir.dt.float32
BF16 = mybir.dt.bfloat16
AX = mybir.AxisListType
OP = mybir.AluOpType
AF = mybir.ActivationFunctionType

B, D, N, L, H, P = 32, 64, 254, 16, 128, 96
N_CORES = 8
B_PER = B // N_CORES          # 4 batches per core
S = B_PER * D                 # 256 slices per core
TILE_S = 128                  # slices per SBUF tile (partition dim)
NT = S // TILE_S              # 2 tiles per core
NL = N * L                    # 4064 f32 per slice

BF16_PROD = True              # bf16 for the two big elementwise products
XS_DVE_LS = 5                 # xs l-planes reduced on DVE (rest on ScalarE)

PDT = BF16 if BF16_PROD else F32


def build_nc():
    nc = bacc.Bacc(
        "TRN2", target_bir_lowering=False, debug=False, num_devices=N_CORES
    )
    x_dram = nc.dram_tensor("x_sh", [S, NL], F32, kind="ExternalInput").ap()
    c_dram = nc.dram_tensor("consts", [128, 754], F32, kind="ExternalInput").ap()
    o_dram = nc.dram_tensor(
        "out_sh", [B_PER, P, D], F32, kind="ExternalOutput"
    ).ap()

    with tile.TileContext(nc) as tc:
        with (
            tc.tile_pool(name="cpool", bufs=1) as cpool,
            tc.tile_pool(name="xpool", bufs=2) as xpool,
            tc.tile_pool(name="work", bufs=2) as work,
            tc.tile_pool(name="sm", bufs=2) as sm,
            tc.tile_pool(name="psum", bufs=1, space="PSUM") as psum,
        ):
            cb = cpool.tile([128, 754], F32)
            nc.sync.dma_start(cb[:], c_dram[:, :])
            ident = cb[:, 0:128]
            peT = cb[:, 128:382]           # (pos_embed + b_enc).T
            wv = cb[:, 382:398]            # W_enc.T
            cbias = cb[:, 398:399]         # N*b_enc + pos_embed.sum(0)
            w2 = cb[:, 400:496]
            b1c = cb[:, 496:497]
            b2c = cb[0:P, 497:498]
            wenc = cb[0:L, 498:626]
            w1 = cb[0:L, 626:754]

            for t in range(NT):
                x_t = xpool.tile([TILE_S, NL], F32)
                nc.sync.dma_start(x_t[:], x_dram[bass.ts(t, TILE_S), :])
                x_nl = x_t[:].rearrange("p (n l) -> p n l", n=N, l=L)
                x_ln = x_t[:].rearrange("p (n l) -> p l n", n=N, l=L)

                if BF16_PROD:
                    x_b = xpool.tile([TILE_S, NL], BF16)
                    nc.scalar.copy(x_b[:, 0 : NL // 2], x_t[:, 0 : NL // 2])
                    nc.scalar.copy(x_b[:, NL // 2 : NL], x_t[:, NL // 2 : NL])
                    xb_nl = x_b[:].rearrange("p (n l) -> p n l", n=N, l=L)
                else:
                    xb_nl = x_nl

                # xs[s, l] = sum_n x[s, n, l]: a few l-planes on DVE (one
                # strided reduce), the rest on ScalarE (act+accum per plane)
                xs = sm.tile([TILE_S, L], F32)
                nc.vector.tensor_reduce(
                    xs[:, 0:XS_DVE_LS], x_ln[:, 0:XS_DVE_LS, :],
                    axis=AX.X, op=OP.add,
                )
                xscr = sm.tile([TILE_S, N], F32)
                for l in range(XS_DVE_LS, L):
                    nc.scalar.activation(
                        xscr[:], x_nl[:, :, l], AF.Copy,
                        accum_out=xs[:, l : l + 1],
                    )

                # xsT[l, s] via PE transpose; csumT = W_enc.T @ xsT + cbias
                xsT_ps = psum.tile([L, TILE_S], F32)
                nc.tensor.transpose(xsT_ps[:], xs[:], ident)
                xsT = sm.tile([L, TILE_S], F32)
                nc.vector.tensor_copy(xsT[:], xsT_ps[:])

                csum_ps = psum.tile([H, TILE_S], F32)
                nc.tensor.matmul(csum_ps[:], wenc, xsT[:])
                csumT = sm.tile([H, TILE_S], F32)
                nc.vector.tensor_scalar_add(csumT[:], csum_ps[:], cbias)

                # v[s, l] = (W_enc @ csum_s)[l];  spe[s, n] = (pe_n+b_enc).csum_s
                vb_ps = psum.tile([TILE_S, L], F32)
                nc.tensor.matmul(vb_ps[:], csumT[:], wv)
                vb = sm.tile([TILE_S, L], PDT)
                nc.vector.tensor_copy(vb[:], vb_ps[:])

                spe_ps = psum.tile([TILE_S, N], F32)
                nc.tensor.matmul(spe_ps[:], csumT[:], peT)

                # sdot[s, n] = x_n . v_s  (product in bf16, 2x DVE mode)
                prod = work.tile([TILE_S, N, L], PDT)
                v_bc = vb[:].unsqueeze(1).broadcast_to((TILE_S, N, L))
                nc.vector.tensor_mul(prod[:], xb_nl, v_bc)
                sdot = sm.tile([TILE_S, N], F32)
                nc.vector.tensor_reduce(sdot[:], prod[:], axis=AX.X, op=OP.add)

                # score = sdot + spe ; weight = score / sum_n |score|
                score = sm.tile([TILE_S, N], F32)
                nc.vector.tensor_add(score[:], sdot[:], spe_ps[:])
                sabs = sm.tile([TILE_S, 1], F32)
                nc.vector.tensor_reduce(
                    sabs[:], score[:], axis=AX.X, op=OP.add,
                    apply_absolute_value=True,
                )
                rec = sm.tile([TILE_S, 1], F32)
                nc.vector.reciprocal(rec[:], sabs[:])
                weight = sm.tile([TILE_S, N], PDT)
                nc.vector.tensor_scalar_mul(weight[:], score[:], rec[:])

                # pooled[s, l] = sum_n weight[s, n] * x[s, n, l]
                prod2 = work.tile([TILE_S, N, L], PDT)
                w_bc = weight[:].unsqueeze(2).broadcast_to((TILE_S, N, L))
                nc.vector.tensor_mul(prod2[:], xb_nl, w_bc)
                pooled = sm.tile([TILE_S, L], F32)
                p2_ln = prod2[:].transpose([0, 2, 1])
                nc.vector.tensor_reduce(pooled[:], p2_ln, axis=AX.X, op=OP.add)

                # FF: h = leaky_relu(pooled @ W1 + b1, 0.2); out = h @ W2 + b2
                pT_ps = psum.tile([L, TILE_S], F32)
                nc.tensor.transpose(pT_ps[:], pooled[:], ident)
                pT = sm.tile([L, TILE_S], F32)
                nc.scalar.copy(pT[:], pT_ps[:])

                h_ps = psum.tile([H, TILE_S], F32)
                nc.tensor.matmul(h_ps[:], w1, pT[:])
                hb = sm.tile([H, TILE_S], F32)
                nc.scalar.activation(hb[:], h_ps[:], AF.Identity, bias=b1c)
                h_sb = sm.tile([H, TILE_S], F32)
                nc.vector.scalar_tensor_tensor(
                    h_sb[:], hb[:], 0.2, hb[:], op0=OP.mult, op1=OP.max
                )

                o_ps = psum.tile([P, TILE_S], F32)
                nc.tensor.matmul(o_ps[:], w2, h_sb[:])
                o_sb = sm.tile([P, TILE_S], F32)
                nc.scalar.activation(o_sb[:], o_ps[:], AF.Identity, bias=b2c)

                for k in range(TILE_S // D):
                    b_loc = t * (TILE_S // D) + k
                    nc.sync.dma_start(o_dram[b_loc], o_sb[:, bass.ts(k, D)])

    nc.compile()
    return nc


_NC_CACHE = None


def _get_nc():
    global _NC_CACHE
    if _NC_CACHE is None:
        _NC_CACHE = build_nc()
    return _NC_CACHE


def _make_consts(W_enc, b_enc, W1, b1, W2, b2, pos_embed):
    cb = np.zeros((128, 754), dtype=np.float32)
    cb[:, 0:128] = np.eye(128, dtype=np.float32)
    cb[:, 128:382] = (pos_embed + b_enc).T
    cb[:, 382:398] = W_enc.T
    cb[:, 398] = N * b_enc + pos_embed.sum(axis=0)
    cb[:, 400:496] = W2
    cb[:, 496] = b1
    cb[0:P, 497] = b2
    cb[0:L, 498:626] = W_enc
    cb[0:L, 626:754] = W1
    return cb


def kernel(x, W_enc, b_enc, W1, b1, W2, b2, pos_embed):
    x = np.ascontiguousarray(np.asarray(x, dtype=np.float32))
    consts = _make_consts(
        np.asarray(W_enc, np.float32), np.asarray(b_enc, np.float32),
        np.asarray(W1, np.float32), np.asarray(b1, np.float32),
        np.asarray(W2, np.float32), np.asarray(b2, np.float32),
        np.asarray(pos_embed, np.float32),
    )
    nc = _get_nc()
    in_maps = []
    for c in range(N_CORES):
        x_sh = x[c * B_PER : (c + 1) * B_PER].reshape(S, NL)
        in_maps.append({"x_sh": np.ascontiguousarray(x_sh), "consts": consts})
    res = run_bass_kernel_spmd(nc, in_maps, list(range(N_CORES)))
    out = np.concatenate(
        [res.results[c]["out_sh"] for c in range(N_CORES)], axis=0
    )  # (B, P, D)
    return out, x


# revision 4
# speedup vs baseline: 1.2567x; 1.2567x over previous
"""Trainium2 Bass kernel for the patch-pooling + FF model.

Reference math (per (b, d) slice, x_s = x[b, d] of shape (N=254, L=16)):
    c      = x_s @ W_enc + b_enc + pos_embed          # (N, H)
    csum   = sum_n c                                  # (H,)
    score  = c @ csum (/ sqrt(H), cancels)            # (N,)
    weight = score / sum_n |score|                    # (N,)
    pooled = x_s.T @ weight                           # (L,)
    h      = leaky_relu(pooled @ W1 + b1, 0.2)        # (H,)
    out    = h @ W2 + b2                              # (P,)
returns (out.transpose(0, 2, 1), x)

Key algebra: score_n = c_n . csum = x_n . (W_enc @ csum)
                     + (pos_embed_n + b_enc) . csum
so c (B,D,N,H) is never materialized. b_enc rides inside pos_embedT.

Sharding: data-parallel over batch, 4 batches (256 (b,d) slices) per core.
Layout: 128 slices on partitions, (n, l) = 4064 f32 on the free dim.
GPSIMD is avoided entirely (it shares SBUF ports with DVE - exclusive
lock, so "offloading" there just stalls DVE). Work splits DVE/ScalarE:
ScalarE casts x to bf16 and sums most xs l-planes via activation+accum;
DVE runs the bf16 products (2x mode) and the 1x reduces; PE does the
small matmuls.
"""

import numpy as np

import concourse.bacc as bacc
import concourse.bass as bass
import concourse.tile as tile
from concourse import mybir
from concourse.bass_utils import run_bass_kernel_spmd

F32 = mybir.dt.float32
BF16 = mybir.dt.bfloat16
AX = mybir.AxisListType
OP = mybir.AluOpType
AF = mybir.ActivationFunctionType

B, D, N, L, H, P = 32, 64, 254, 16, 128, 96
N_CORES = 8
B_PER = B // N_CORES          # 4 batches per core
S = B_PER * D                 # 256 slices per core
TILE_S = 128                  # slices per SBUF tile (partition dim)
NT = S // TILE_S              # 2 tiles per core
NL = N * L                    # 4064 f32 per slice

BF16_PROD = True              # bf16 for the two big elementwise products
XS_DVE_LS = 5                 # xs l-planes reduced on DVE (rest on ScalarE)

PDT = BF16 if BF16_PROD else F32


def build_nc():
    nc = bacc.Bacc(
        "TRN2", target_bir_lowering=False, debug=False, num_devices=N_CORES
    )
    x_dram = nc.dram_tensor("x_sh", [S, NL], F32, kind="ExternalInput").ap()
    c_dram = nc.dram_tensor("consts", [128, 754], F32, kind="ExternalInput").ap()
    o_dram = nc.dram_tensor(
        "out_sh", [B_PER, P, D], F32, kind="ExternalOutput"
    ).ap()

    with tile.TileContext(nc) as tc:
        with (
            tc.tile_pool(name="cpool", bufs=1) as cpool,
            tc.tile_pool(name="xpool", bufs=2) as xpool,
            tc.tile_pool(name="work", bufs=2) as work,
            tc.tile_pool(name="sm", bufs=2) as sm,
            tc.tile_pool(name="psum", bufs=1, space="PSUM") as psum,
        ):
            cb = cpool.tile([128, 754], F32)
            nc.sync.dma_start(cb[:], c_dram[:, :])
            ident = cb[:, 0:128]
            peT = cb[:, 128:382]           # (pos_embed + b_enc).T
            wv = cb[:, 382:398]            # W_enc.T
            cbias = cb[:, 398:399]         # N*b_enc + pos_embed.sum(0)
            w2 = cb[:, 400:496]
            b1c = cb[:, 496:497]
            b2c = cb[0:P, 497:498]
            wenc = cb[0:L, 498:626]
            w1 = cb[0:L, 626:754]

            NH = N // 2  # 127
            for t in range(NT):
                x_t = xpool.tile([TILE_S, NL], F32)
                # halves so downstream work can start after the first lands
                nc.sync.dma_start(
                    x_t[:, 0 : NH * L], x_dram[bass.ts(t, TILE_S), 0 : NH * L]
                )
                nc.sync.dma_start(
                    x_t[:, NH * L : NL], x_dram[bass.ts(t, TILE_S), NH * L : NL]
                )
                x_nl = x_t[:].rearrange("p (n l) -> p n l", n=N, l=L)
                x_ln = x_t[:].rearrange("p (n l) -> p l n", n=N, l=L)

                if BF16_PROD:
                    x_b = xpool.tile([TILE_S, 4096 * NT], BF16)
                    nc.scalar.copy(x_b[:, 0 : NL // 2], x_t[:, 0 : NL // 2])
                    nc.scalar.copy(x_b[:, NL // 2 : NL], x_t[:, NL // 2 : NL])
                    xb_nl = x_b[:, 0:NL].rearrange("p (n l) -> p n l", n=N, l=L)
                else:
                    xb_nl = x_nl

                # xs[s, l] = sum_n x[s, n, l]
                # tile 0 gates everything: both n-halves on DVE (fast, can
                # start on the first DMA half). tile 1 overlaps tile 0's
                # DVE body: most planes on ScalarE.
                xs = sm.tile([TILE_S, L], F32)
                if t == 0:
                    xsh = sm.tile([TILE_S, 2, L], F32)
                    nc.vector.tensor_reduce(
                        xsh[:, 0, :], x_ln[:, :, 0:NH], axis=AX.X, op=OP.add
                    )
                    nc.vector.tensor_reduce(
                        xsh[:, 1, :], x_ln[:, :, NH:N], axis=AX.X, op=OP.add
                    )
                    nc.vector.tensor_add(xs[:], xsh[:, 0, :], xsh[:, 1, :])
                else:
                    nc.vector.tensor_reduce(
                        xs[:, 0:XS_DVE_LS], x_ln[:, 0:XS_DVE_LS, :],
                        axis=AX.X, op=OP.add,
                    )
                    xscr = sm.tile([TILE_S, N], F32)
                    for l in range(XS_DVE_LS, L):
                        nc.scalar.activation(
                            xscr[:], x_nl[:, :, l], AF.Copy,
                            accum_out=xs[:, l : l + 1],
                        )

                # xsT[l, s] via PE transpose; csumT = W_enc.T @ xsT + cbias
                xsT_ps = psum.tile([L, TILE_S], F32)
                nc.tensor.transpose(xsT_ps[:], xs[:], ident)
                xsT = sm.tile([L, TILE_S], F32)
                nc.vector.tensor_copy(xsT[:], xsT_ps[:])

                csum_ps = psum.tile([H, TILE_S], F32)
                nc.tensor.matmul(csum_ps[:], wenc, xsT[:])
                csumT = sm.tile([H, TILE_S], F32)
                nc.vector.tensor_scalar_add(csumT[:], csum_ps[:], cbias)

                # v[s, l] = (W_enc @ csum_s)[l];  spe[s, n] = (pe_n+b_enc).csum_s
                vb_ps = psum.tile([TILE_S, L], F32)
                nc.tensor.matmul(vb_ps[:], csumT[:], wv)
                vb = sm.tile([TILE_S, L], PDT)
                nc.vector.tensor_copy(vb[:], vb_ps[:])

                spe_ps = psum.tile([TILE_S, N], F32)
                nc.tensor.matmul(spe_ps[:], csumT[:], peT)

                # sdot[s, n] = x_n . v_s  (product in bf16, 2x DVE mode)
                prod = work.tile([TILE_S, N, L], PDT)
                v_bc = vb[:].unsqueeze(1).broadcast_to((TILE_S, N, L))
                nc.vector.tensor_mul(prod[:], xb_nl, v_bc)
                sdot = sm.tile([TILE_S, N], F32)
                nc.vector.tensor_reduce(sdot[:], prod[:], axis=AX.X, op=OP.add)

                # score = sdot + spe ; weight = score / sum_n |score|
                score = sm.tile([TILE_S, N], F32)
                nc.vector.tensor_add(score[:], sdot[:], spe_ps[:])
                sabs = sm.tile([TILE_S, 1], F32)
                nc.vector.tensor_reduce(
                    sabs[:], score[:], axis=AX.X, op=OP.add,
                    apply_absolute_value=True,
                )
                rec = sm.tile([TILE_S, 1], F32)
                nc.vector.reciprocal(rec[:], sabs[:])
                weight = sm.tile([TILE_S, N], PDT)
                nc.vector.tensor_scalar_mul(weight[:], score[:], rec[:])

                # pooled[s, l] = sum_n weight[s, n] * x[s, n, l]
                # contiguous fold-tree over n (strided reduce is ~1.6x slower)
                prod2 = work.tile([TILE_S, 4096, 1], PDT)
                p2v = prod2[:, 0:NL, 0].rearrange("p (n l) -> p n l", n=N, l=L)
                w_bc = weight[:].unsqueeze(2).broadcast_to((TILE_S, N, L))
                nc.vector.tensor_mul(p2v, xb_nl, w_bc)
                fold = work.tile([TILE_S, NH * L], F32)
                nc.vector.tensor_add(
                    fold[:], p2v[:, 0:NH, :], p2v[:, NH:N, :]
                )
                n_cur = NH  # 127
                while n_cur > 1:
                    h = n_cur // 2
                    odd = n_cur - 2 * h
                    nc.vector.tensor_add(
                        fold[:, 0 : h * L],
                        fold[:, 0 : h * L],
                        fold[:, h * L : 2 * h * L],
                    )
                    if odd:
                        nc.vector.tensor_add(
                            fold[:, 0:L],
                            fold[:, 0:L],
                            fold[:, 2 * h * L : n_cur * L],
                        )
                    n_cur = h
                pooled = sm.tile([TILE_S, L], F32)
                nc.vector.tensor_copy(pooled[:], fold[:, 0:L])

                # FF: h = leaky_relu(pooled @ W1 + b1, 0.2); out = h @ W2 + b2
                pT_ps = psum.tile([L, TILE_S], F32)
                nc.tensor.transpose(pT_ps[:], pooled[:], ident)
                pT = sm.tile([L, TILE_S], F32)
                nc.scalar.copy(pT[:], pT_ps[:])

                h_ps = psum.tile([H, TILE_S], F32)
                nc.tensor.matmul(h_ps[:], w1, pT[:])
                hb = sm.tile([H, TILE_S], F32)
                nc.scalar.activation(hb[:], h_ps[:], AF.Identity, bias=b1c)
                h_sb = sm.tile([H, TILE_S], F32)
                nc.vector.scalar_tensor_tensor(
                    h_sb[:], hb[:], 0.2, hb[:], op0=OP.mult, op1=OP.max
                )

                o_ps = psum.tile([P, TILE_S], F32)
                nc.tensor.matmul(o_ps[:], w2, h_sb[:])
                o_sb = sm.tile([P, TILE_S], F32)
                nc.scalar.activation(o_sb[:], o_ps[:], AF.Identity, bias=b2c)

                for k in range(TILE_S // D):
                    b_loc = t * (TILE_S // D) + k
                    nc.sync.dma_start(o_dram[b_loc], o_sb[:, bass.ts(k, D)])

    nc.compile()
    return nc


_NC_CACHE = None


def _get_nc():
    global _NC_CACHE
    if _NC_CACHE is None:
        _NC_CACHE = build_nc()
    return _NC_CACHE


def _make_consts(W_enc, b_enc, W1, b1, W2, b2, pos_embed):
    cb = np.zeros((128, 754), dtype=np.float32)
    cb[:, 0:128] = np.eye(128, dtype=np.float32)
    cb[:, 128:382] = (pos_embed + b_enc).T
    cb[:, 382:398] = W_enc.T
    cb[:, 398] = N * b_enc + pos_embed.sum(axis=0)
    cb[:, 400:496] = W2
    cb[:, 496] = b1
    cb[0:P, 497] = b2
    cb[0:L, 498:626] = W_enc
    cb[0:L, 626:754] = W1
    return cb


def kernel(x, W_enc, b_enc, W1, b1, W2, b2, pos_embed):
    x = np.ascontiguousarray(np.asarray(x, dtype=np.float32))
    consts = _make_consts(
        np.asarray(W_enc, np.float32), np.asarray(b_enc, np.float32),
        np.asarray(W1, np.float32), np.asarray(b1, np.float32),
        np.asarray(W2, np.float32), np.asarray(b2, np.float32),
        np.asarray(pos_embed, np.float32),
    )
    nc = _get_nc()
    in_maps = []
    for c in range(N_CORES):
        x_sh = x[c * B_PER : (c + 1) * B_PER].reshape(S, NL)
        in_maps.append({"x_sh": np.ascontiguousarray(x_sh), "consts": consts})
    res = run_bass_kernel_spmd(nc, in_maps, list(range(N_CORES)))
    out = np.concatenate(
        [res.results[c]["out_sh"] for c in range(N_CORES)], axis=0
    )  # (B, P, D)
    return out, x


# revision 5
# speedup vs baseline: 1.3329x; 1.0607x over previous
"""Trainium2 Bass kernel for the patch-pooling + FF model.

Reference math (per (b, d) slice, x_s = x[b, d] of shape (N=254, L=16)):
    c      = x_s @ W_enc + b_enc + pos_embed          # (N, H)
    csum   = sum_n c                                  # (H,)
    score  = c @ csum (/ sqrt(H), cancels)            # (N,)
    weight = score / sum_n |score|                    # (N,)
    pooled = x_s.T @ weight                           # (L,)
    h      = leaky_relu(pooled @ W1 + b1, 0.2)        # (H,)
    out    = h @ W2 + b2                              # (P,)
returns (out.transpose(0, 2, 1), x)

Key algebra: score_n = c_n . csum = x_n . (W_enc @ csum)
                     + (pos_embed_n + b_enc) . csum
so c (B,D,N,H) is never materialized. b_enc rides inside pos_embedT.

Sharding: data-parallel over batch, 4 batches (256 (b,d) slices) per core.
Layout: 128 slices on partitions, (n, l) = 4064 f32 on the free dim.
GPSIMD is avoided entirely (it shares SBUF ports with DVE - exclusive
lock, so "offloading" there just stalls DVE). Work splits DVE/ScalarE:
ScalarE casts x to bf16 and sums most xs l-planes via activation+accum;
DVE runs the bf16 products (2x mode) and the 1x reduces; PE does the
small matmuls.
"""

import numpy as np

import concourse.bacc as bacc
import concourse.bass as bass
import concourse.tile as tile
from concourse import mybir
from concourse.bass_utils import run_bass_kernel_spmd

F32 = mybir.dt.float32
BF16 = mybir.dt.bfloat16
AX = mybir.AxisListType
OP = mybir.AluOpType
AF = mybir.ActivationFunctionType

B, D, N, L, H, P = 32, 64, 254, 16, 128, 96
N_CORES = 8
B_PER = B // N_CORES          # 4 batches per core
S = B_PER * D                 # 256 slices per core
TILE_S = 128                  # slices per SBUF tile (partition dim)
NT = S // TILE_S              # 2 tiles per core
NL = N * L                    # 4064 f32 per slice

BF16_PROD = True              # bf16 for the two big elementwise products
XS_DVE_LS = 5                 # xs l-planes reduced on DVE (rest on ScalarE)

PDT = BF16 if BF16_PROD else F32


def build_nc():
    nc = bacc.Bacc(
        "TRN2", target_bir_lowering=False, debug=False, num_devices=N_CORES
    )
    x_dram = nc.dram_tensor("x_sh", [S, NL], F32, kind="ExternalInput").ap()
    c_dram = nc.dram_tensor("consts", [128, 754], F32, kind="ExternalInput").ap()
    o_dram = nc.dram_tensor(
        "out_sh", [B_PER, P, D], F32, kind="ExternalOutput"
    ).ap()

    with tile.TileContext(nc) as tc:
        with (
            tc.tile_pool(name="cpool", bufs=1) as cpool,
            tc.tile_pool(name="xpool", bufs=2) as xpool,
            tc.tile_pool(name="work", bufs=2) as work,
            tc.tile_pool(name="sm", bufs=2) as sm,
            tc.tile_pool(name="psum", bufs=1, space="PSUM") as psum,
        ):
            cb = cpool.tile([128, 754], F32)
            ident = cb[:, 0:128]
            peT = cb[:, 128:382]           # (pos_embed + b_enc).T
            wv = cb[:, 382:398]            # W_enc.T
            cbias = cb[:, 398:399]         # N*b_enc + pos_embed.sum(0)
            w2 = cb[:, 400:496]
            b1c = cb[:, 496:497]
            b2c = cb[0:P, 497:498]
            wenc = cb[0:L, 498:626]
            w1 = cb[0:L, 626:754]

            NH = N // 2  # 127
            HL = NH * L  # 2032
            xh = [[None, None], [None, None]]   # [t][half] f32 tiles
            xbh = [[None, None], [None, None]]  # [t][half] bf16 tiles
            # issue all x DMAs first (consts only gate the PE chain)
            for t in range(NT):
                for hf in range(2):
                    xt = xpool.tile([TILE_S, HL], F32, name=f"x{t}{hf}")
                    nc.sync.dma_start(
                        xt[:],
                        x_dram[bass.ts(t, TILE_S), hf * HL : (hf + 1) * HL],
                    )
                    xh[t][hf] = xt
            nc.sync.dma_start(cb[:], c_dram[:, :])

            for t in range(NT):
                x_ta, x_tb = xh[t]
                xa_nl = x_ta[:].rearrange("p (n l) -> p n l", n=NH, l=L)
                xb2_nl = x_tb[:].rearrange("p (n l) -> p n l", n=NH, l=L)
                xa_ln = x_ta[:].rearrange("p (n l) -> p l n", n=NH, l=L)
                xb2_ln = x_tb[:].rearrange("p (n l) -> p l n", n=NH, l=L)

                xba = xpool.tile([TILE_S, HL], BF16, name=f"xb{t}0")
                nc.scalar.copy(xba[:], x_ta[:])
                xbb = xpool.tile([TILE_S, HL], BF16, name=f"xb{t}1")
                nc.scalar.copy(xbb[:], x_tb[:])
                xbh[t] = [xba, xbb]

                # xs[s, l] = sum_n x[s, n, l]
                # tile 0 gates everything: both halves on DVE.
                # tile 1: half a on DVE, half b on ScalarE planes
                # (overlapping tile 0's DVE body).
                xs = sm.tile([TILE_S, L], F32)
                xsh = sm.tile([TILE_S, 2, L], F32)
                nc.vector.tensor_reduce(
                    xsh[:, 0, :], xa_ln, axis=AX.X, op=OP.add
                )
                if t == 0:
                    nc.vector.tensor_reduce(
                        xsh[:, 1, :], xb2_ln, axis=AX.X, op=OP.add
                    )
                else:
                    xscr = sm.tile([TILE_S, NH], F32)
                    for l in range(L):
                        nc.scalar.activation(
                            xscr[:], xb2_nl[:, :, l], AF.Copy,
                            accum_out=xsh[:, 1, l : l + 1],
                        )
                nc.vector.tensor_add(xs[:], xsh[:, 0, :], xsh[:, 1, :])

                # xsT[l, s] via PE transpose; csumT = W_enc.T @ xsT + cbias
                xsT_ps = psum.tile([L, TILE_S], F32)
                nc.tensor.transpose(xsT_ps[:], xs[:], ident)
                xsT = sm.tile([L, TILE_S], F32)
                nc.vector.tensor_copy(xsT[:], xsT_ps[:])

                csum_ps = psum.tile([H, TILE_S], F32)
                nc.tensor.matmul(csum_ps[:], wenc, xsT[:])
                csumT = sm.tile([H, TILE_S], F32)
                nc.vector.tensor_scalar_add(csumT[:], csum_ps[:], cbias)

                # v[s, l] = (W_enc @ csum_s)[l];  spe[s, n] = (pe_n+b_enc).csum_s
                vb_ps = psum.tile([TILE_S, L], F32)
                nc.tensor.matmul(vb_ps[:], csumT[:], wv)
                vb = sm.tile([TILE_S, L], PDT)
                nc.vector.tensor_copy(vb[:], vb_ps[:])

                spe_ps = psum.tile([TILE_S, N], F32)
                nc.tensor.matmul(spe_ps[:], csumT[:], peT)

                # sdot[s, n] = x_n . v_s  (bf16 products, 2x DVE mode)
                v_bc = vb[:].unsqueeze(1).broadcast_to((TILE_S, NH, L))
                sdot = sm.tile([TILE_S, N], F32)
                proda = work.tile([TILE_S, NH, L], PDT, name="proda")
                nc.vector.tensor_mul(proda[:], xbh[t][0][:].rearrange(
                    "p (n l) -> p n l", n=NH, l=L), v_bc)
                nc.vector.tensor_reduce(
                    sdot[:, 0:NH], proda[:], axis=AX.X, op=OP.add)
                prodb = work.tile([TILE_S, NH, L], PDT, name="prodb")
                nc.vector.tensor_mul(prodb[:], xbh[t][1][:].rearrange(
                    "p (n l) -> p n l", n=NH, l=L), v_bc)
                nc.vector.tensor_reduce(
                    sdot[:, NH:N], prodb[:], axis=AX.X, op=OP.add)

                # score = sdot + spe ; weight = score / sum_n |score|
                score = sm.tile([TILE_S, N], F32)
                nc.vector.tensor_add(score[:], sdot[:], spe_ps[:])
                sabs = sm.tile([TILE_S, 1], F32)
                nc.vector.tensor_reduce(
                    sabs[:], score[:], axis=AX.X, op=OP.add,
                    apply_absolute_value=True,
                )
                rec = sm.tile([TILE_S, 1], F32)
                nc.vector.reciprocal(rec[:], sabs[:])
                # weight as [p, N, 2] pairs so the bf16 product below reads
                # contiguous 4B pairs (keeps the DVE 2x mode; a plain
                # stride-0 innermost broadcast drops to 1x)
                weight = sm.tile([TILE_S, N, 2], PDT)
                nc.vector.tensor_scalar_mul(
                    weight[:],
                    score[:].unsqueeze(2).broadcast_to((TILE_S, N, 2)),
                    rec[:],
                )

                # pooled[s, l] = sum_n weight[s, n] * x[s, n, l]
                # products per half, then a contiguous fold-tree over n
                fold = work.tile([TILE_S, HL], F32)
                L2 = L // 2
                for hf in range(2):
                    p2 = work.tile([TILE_S, NH, L2, 2], PDT, name=f"p2{hf}")
                    w_bc = weight[:, bass.ds(hf * NH, NH), :].unsqueeze(
                        2).broadcast_to((TILE_S, NH, L2, 2))
                    xb4 = xbh[t][hf][:].rearrange(
                        "p (n a b) -> p n a b", n=NH, a=L2, b=2)
                    nc.vector.tensor_mul(p2[:], xb4, w_bc)
                    if hf == 0:
                        p20 = p2
                    else:
                        nc.vector.tensor_add(
                            fold[:],
                            p20[:].rearrange("p n a b -> p (n a b)"),
                            p2[:].rearrange("p n a b -> p (n a b)"),
                        )
                n_cur = NH  # 127
                while n_cur > 1:
                    h = n_cur // 2
                    odd = n_cur - 2 * h
                    nc.vector.tensor_add(
                        fold[:, 0 : h * L],
                        fold[:, 0 : h * L],
                        fold[:, h * L : 2 * h * L],
                    )
                    if odd:
                        nc.vector.tensor_add(
                            fold[:, 0:L],
                            fold[:, 0:L],
                            fold[:, 2 * h * L : n_cur * L],
                        )
                    n_cur = h
                pooled = sm.tile([TILE_S, L], F32)
                nc.vector.tensor_copy(pooled[:], fold[:, 0:L])

                # FF: h = leaky_relu(pooled @ W1 + b1, 0.2); out = h @ W2 + b2
                pT_ps = psum.tile([L, TILE_S], F32)
                nc.tensor.transpose(pT_ps[:], pooled[:], ident)
                pT = sm.tile([L, TILE_S], F32)
                nc.scalar.copy(pT[:], pT_ps[:])

                h_ps = psum.tile([H, TILE_S], F32)
                nc.tensor.matmul(h_ps[:], w1, pT[:])
                hb = sm.tile([H, TILE_S], F32)
                nc.scalar.activation(hb[:], h_ps[:], AF.Identity, bias=b1c)
                h_sb = sm.tile([H, TILE_S], F32)
                nc.vector.scalar_tensor_tensor(
                    h_sb[:], hb[:], 0.2, hb[:], op0=OP.mult, op1=OP.max
                )

                o_ps = psum.tile([P, TILE_S], F32)
                nc.tensor.matmul(o_ps[:], w2, h_sb[:])
                o_sb = sm.tile([P, TILE_S], F32)
                nc.scalar.activation(o_sb[:], o_ps[:], AF.Identity, bias=b2c)

                for k in range(TILE_S // D):
                    b_loc = t * (TILE_S // D) + k
                    nc.sync.dma_start(o_dram[b_loc], o_sb[:, bass.ts(k, D)])

    nc.compile()
    return nc


_NC_CACHE = None


def _get_nc():
    global _NC_CACHE
    if _NC_CACHE is None:
        _NC_CACHE = build_nc()
    return _NC_CACHE


def _make_consts(W_enc, b_enc, W1, b1, W2, b2, pos_embed):
    cb = np.zeros((128, 754), dtype=np.float32)
    cb[:, 0:128] = np.eye(128, dtype=np.float32)
    cb[:, 128:382] = (pos_embed + b_enc).T
    cb[:, 382:398] = W_enc.T
    cb[:, 398] = N * b_enc + pos_embed.sum(axis=0)
    cb[:, 400:496] = W2
    cb[:, 496] = b1
    cb[0:P, 497] = b2
    cb[0:L, 498:626] = W_enc
    cb[0:L, 626:754] = W1
    return cb


def kernel(x, W_enc, b_enc, W1, b1, W2, b2, pos_embed):
    x = np.ascontiguousarray(np.asarray(x, dtype=np.float32))
    consts = _make_consts(
        np.asarray(W_enc, np.float32), np.asarray(b_enc, np.float32),
        np.asarray(W1, np.float32), np.asarray(b1, np.float32),
        np.asarray(W2, np.float32), np.asarray(b2, np.float32),
        np.asarray(pos_embed, np.float32),
    )
    nc = _get_nc()
    in_maps = []
    for c in range(N_CORES):
        x_sh = x[c * B_PER : (c + 1) * B_PER].reshape(S, NL)
        in_maps.append({"x_sh": np.ascontiguousarray(x_sh), "consts": consts})
    res = run_bass_kernel_spmd(nc, in_maps, list(range(N_CORES)))
    out = np.concatenate(
        [res.results[c]["out_sh"] for c in range(N_CORES)], axis=0
    )  # (B, P, D)
    return out, x
